# revision 2
# baseline (speedup 1.0000x reference)
"""BinaryMLP (nn_BinaryMLP_91276644974884) on 8 TRN2 NeuronCores — v2.

Reference network (B=32768, D=784, H1=H2=4096, C=10):
    h  = x @ W1.T + b1                    # fc1
    h  = BN1(prelu(h, a1)) (batch stats)
    h  = sign(h) @ sign(W2).T             # fc2, binary GEMM
    h  = BN2(prelu(h, a2))
    o  = log_softmax(h @ W3.T + b3)

Data-parallel over batch (4096 rows/core), [features, batch] layout.

v3: baseline-exact fc1 (19-stream fp16 hi/lo, fp32 p1 DRAM bounce,
error ~2e-7) + fused phase 2: fc2 DoubleRow binary GEMM with p2 kept in
a 6-deep SBUF window, BN2-affine + fc3 [10,512] matmuls accumulated in
PSUM per stats-group and added into a persistent logits tile, tight
log_softmax epilogue.  Eliminates the entire separate phase 3 and the
p2 DRAM round trip.
"""

import numpy as np
import ml_dtypes

import concourse.bass as bass
import concourse.tile as tile
from concourse import bacc, mybir
from concourse.bass_utils import run_bass_kernel_spmd

F32 = mybir.dt.float32
F32R = mybir.dt.float32r
F16 = mybir.dt.float16
F8 = mybir.dt.float8e4
AF = mybir.ActivationFunctionType
ALU = mybir.AluOpType

NCORES = 8
B = 32768
BS = B // NCORES          # 4096 batch rows per core
D = 784
K1ROWS = 2 * (D + 1) + D  # 2354: [xh+bias; xh+bias; xl] packed along K
KC1 = -(-K1ROWS // 128)   # 19 chunks (padded to 2432)
FS = 2048.0               # 2^11 hi/lo split scale
H1 = 4096
H2 = 4096
MT = 32                   # 4096 / 128 feature tiles
C = 10
NB = BS // 512            # 8 512-col chunks per core
EPS = 1e-5
NG = 8                    # BN stat groups per layer (pipelined AllReduces)
GM = MT // NG             # 4 feature tiles per group
QS = 512                  # sign-pass col chunk (= one mean-centering chunk)
P1W = 8                   # p1 SBUF window depth (tiles of [128, BS] fp16)
P2W = 6                   # p2 SBUF window depth


def build_program(debug=False):
    nc = bacc.Bacc("TRN2", target_bir_lowering=False, debug=False,
                   num_devices=NCORES)

    xT = nc.declare_dram_parameter("xT", [128, NB, KC1, 512], F16,
                                   isOutput=False)
    w1 = nc.declare_dram_parameter("w1", [MT, 128, KC1, 128], F16,
                                   isOutput=False)
    w2 = nc.declare_dram_parameter("w2", [MT, 128, MT, 128], F8, isOutput=False)
    w3 = nc.declare_dram_parameter("w3", [128, MT, C], F16, isOutput=False)
    g1 = nc.declare_dram_parameter("g1", [128, MT], F32, isOutput=False)
    bt1 = nc.declare_dram_parameter("bt1", [128, MT], F32, isOutput=False)
    g2 = nc.declare_dram_parameter("g2", [128, MT], F32, isOutput=False)
    bt2 = nc.declare_dram_parameter("bt2", [128, MT], F32, isOutput=False)
    a1p = nc.declare_dram_parameter("a1p", [128, 1], F32, isOutput=False)
    a2p = nc.declare_dram_parameter("a2p", [128, 1], F32, isOutput=False)
    b3p = nc.declare_dram_parameter("b3p", [C, 1], F32, isOutput=False)
    eye = nc.declare_dram_parameter("eye", [C, C], F32, isOutput=False)
    out = nc.declare_dram_parameter("out", [BS, C], F32, isOutput=True)

    dbg = {}
    if debug:
        for nm, shp in [
            ("dbg_scale1", [128, MT]), ("dbg_bias1", [128, MT]),
            ("dbg_scale2", [128, MT]), ("dbg_bias2", [128, MT]),
            ("dbg_p2", [128, 512]), ("dbg_logits", [128, 2048]),
        ]:
            dbg[nm] = nc.declare_dram_parameter(nm, shp, F32, isOutput=True)

    with tile.TileContext(nc) as tc:
        with (
            tc.tile_pool(name="const", bufs=1) as const_pool,
            tc.tile_pool(name="stats", bufs=1) as stats_pool,
            tc.tile_pool(name="dram", bufs=1, space="DRAM") as dram_pool,
        ):
            # ---- persistent small tiles -------------------------------------
            g1_t = const_pool.tile([128, MT], F32, tag="g1")
            bt1_t = const_pool.tile([128, MT], F32, tag="bt1")
            g2_t = const_pool.tile([128, MT], F32, tag="g2")
            bt2_t = const_pool.tile([128, MT], F32, tag="bt2")
            a1_t = const_pool.tile([128, 1], F32, tag="a1")
            a2_t = const_pool.tile([128, 1], F32, tag="a2")
            b3_t = const_pool.tile([C, 1], F32, tag="b3")
            eye_t = const_pool.tile([C, C], F32, tag="eye")
            w3_t = const_pool.tile([128, MT, C], F16, tag="w3")
            for t, d in [(g1_t, g1), (bt1_t, bt1), (g2_t, g2), (bt2_t, bt2),
                         (a1_t, a1p), (a2_t, a2p), (b3_t, b3p), (eye_t, eye),
                         (w3_t, w3)]:
                nc.sync.dma_start(t[:], d.ap())

            sums1 = stats_pool.tile([128, MT, NB], F32, tag="sums1")
            sq1 = stats_pool.tile([128, MT, NB], F32, tag="sq1")
            sums2 = stats_pool.tile([128, MT, NB], F32, tag="sums2")
            sq2 = stats_pool.tile([128, MT, NB], F32, tag="sq2")

            s1d = dram_pool.tile([MT, 128, BS], F8, tag="s1d")
            p1d = dram_pool.tile([MT, 128, BS], F32, tag="p1d")
            cc_in1 = dram_pool.tile([NG, 128, 2 * GM], F32, tag="cc_in1")
            cc_out1 = dram_pool.tile([NG, 128, 2 * GM], F32, tag="cc_out1")
            cc_in2 = dram_pool.tile([NG, 128, 2 * GM], F32, tag="cc_in2")
            cc_out2 = dram_pool.tile([NG, 128, 2 * GM], F32, tag="cc_out2")

            scale1 = stats_pool.tile([128, MT], F32, tag="scale1")
            bias1 = stats_pool.tile([128, MT], F32, tag="bias1")
            scale2 = stats_pool.tile([128, MT], F32, tag="scale2")
            bias2 = stats_pool.tile([128, MT], F32, tag="bias2")

            def bn_group(sums, sq, cc_in, cc_out, g_t, bt_t, scale, bias,
                         g, tag):
                """Finalize BN scale/bias for feature tiles g*GM..(g+1)*GM-1."""
                msl = slice(g * GM, (g + 1) * GM)
                cat = stats_pool.tile([128, 2 * GM], F32, tag=f"cat{tag}_{g}",
                                      name=f"cat{tag}_{g}")
                nc.vector.reduce_sum(cat[:, 0:GM], sums[:, msl, :],
                                     axis=mybir.AxisListType.X)
                nc.vector.reduce_sum(cat[:, GM:], sq[:, msl, :],
                                     axis=mybir.AxisListType.X)
                nc.sync.dma_start(cc_in[g], cat[:])
                nc.gpsimd.collective_compute(
                    "AllReduce", ALU.add,
                    replica_groups=[list(range(NCORES))],
                    ins=[cc_in[g].opt()], outs=[cc_out[g].opt()],
                )
                red = stats_pool.tile([128, 2 * GM], F32, tag=f"red{tag}_{g}",
                                      name=f"red{tag}_{g}")
                nc.sync.dma_start(red[:], cc_out[g])
                mu = stats_pool.tile([128, GM], F32, tag=f"mu{tag}_{g}",
                                     name=f"mu{tag}_{g}")
                nc.vector.tensor_scalar_mul(mu[:], red[:, 0:GM], 1.0 / B)
                var = stats_pool.tile([128, GM], F32, tag=f"var{tag}_{g}",
                                      name=f"var{tag}_{g}")
                # var = E[p^2] - mu^2 + EPS  (fold the +EPS in here)
                nc.vector.tensor_mul(var[:], mu[:], mu[:])
                nc.vector.scalar_tensor_tensor(
                    var[:], red[:, GM:], 1.0 / B, var[:], ALU.mult, ALU.subtract,
                )
                nc.vector.tensor_scalar_add(var[:], var[:], EPS)
                rinv = stats_pool.tile([128, GM], F32, tag=f"rinv{tag}_{g}",
                                       name=f"rinv{tag}_{g}")
                nc.vector.reciprocal(rinv[:], var[:])
                r = stats_pool.tile([128, GM], F32, tag=f"r{tag}_{g}",
                                    name=f"r{tag}_{g}")
                nc.scalar.activation(r[:], rinv[:], AF.Sqrt)
                nc.vector.tensor_mul(scale[:, msl], g_t[:, msl], r[:])
                nc.vector.tensor_mul(bias[:, msl], mu[:], scale[:, msl])
                nc.vector.tensor_sub(bias[:, msl], bt_t[:, msl], bias[:, msl])

            # ================= Phase 1: fc1 + prelu + stats + sign ===========
            ph1 = nc.named_scope("fc1")
            ph1.__enter__()
            with (
                tc.tile_pool(name="xt", bufs=1) as xt_pool,
                tc.tile_pool(name="w1p", bufs=2) as w1_pool,
                tc.tile_pool(name="p1t", bufs=3) as p1_pool,
                tc.tile_pool(name="scr1", bufs=2) as scr_pool,
                tc.tile_pool(name="pin", bufs=2) as pin_pool,
                tc.tile_pool(name="s1s", bufs=2) as s1s_pool,
                tc.tile_pool(name="ps1", bufs=4, space="PSUM") as ps1_pool,
            ):
                def load_w1(m):
                    w1_t = w1_pool.tile([128, KC1, 128], F16, tag="w1",
                                        name=f"w1_{m}")
                    for k0, k1 in ((0, 10), (10, KC1)):
                        nc.sync.dma_start(
                            w1_t[:, k0:k1, :], w1.ap()[m][:, k0:k1, :]
                        )
                    return w1_t

                w1_cur = load_w1(0)
                xt_t = xt_pool.tile([128, NB, KC1, 512], F16, tag="xt")
                for n in range(NB):
                    for k0, k1 in ((0, 5), (5, 10), (10, 15), (15, KC1)):
                        nc.sync.dma_start(
                            xt_t[:, n, k0:k1, :], xT.ap()[:, n, k0:k1, :]
                        )

                QS1 = 1024
                sign_tasks = []

                def sign_group(g):
                    for mm in range(g * GM, (g + 1) * GM):
                        for q in range(BS // QS1):
                            sign_tasks.append((mm, q))

                def emit_signs(k):
                    for _ in range(min(k, len(sign_tasks))):
                        mm, q = sign_tasks.pop(0)
                        pin = pin_pool.tile([128, QS1], F32, tag="pin",
                                            name=f"pin_{mm}_{q}")
                        nc.gpsimd.dma_start(
                            pin[:], p1d[mm, :, q * QS1:(q + 1) * QS1]
                        )
                        st = s1s_pool.tile([128, QS1], F8, tag="s1s",
                                           name=f"s1s_{mm}_{q}")
                        nc.scalar.activation(
                            st[:], pin[:], AF.Sign,
                            bias=bias1[:, mm:mm + 1], scale=scale1[:, mm:mm + 1],
                        )
                        nc.gpsimd.dma_start(
                            s1d[mm, :, q * QS1:(q + 1) * QS1], st[:]
                        )

                for m in range(MT):
                    emit_signs(3)
                    w1_t = w1_cur
                    if m + 1 < MT:
                        w1_cur = load_w1(m + 1)
                    for n in range(NB):
                        ps = ps1_pool.tile([128, 512], F32, tag="mm")
                        for k in range(KC1):
                            nc.tensor.matmul(
                                ps[:], w1_t[:, k, :], xt_t[:, n, k, :],
                                start=(k == 0), stop=(k == KC1 - 1),
                            )
                        p1_t = p1_pool.tile([128, 512], F32, tag="p1")
                        nc.scalar.activation(
                            p1_t[:], ps[:], AF.Prelu, alpha=a1_t[:],
                            scale=1.0 / FS,
                            accum_out=sums1[:, m, n:n + 1],
                        )
                        scr = scr_pool.tile([128, 512], F16, tag="scr")
                        nc.vector.scalar_tensor_tensor(
                            scr[:], p1_t[:], 0.0, p1_t[:], ALU.add, ALU.mult,
                            accum_out=sq1[:, m, n:n + 1],
                        )
                        nc.sync.dma_start(
                            p1d[m, :, n * 512:(n + 1) * 512], p1_t[:]
                        )
                    if m % GM == GM - 1:
                        bn_group(sums1, sq1, cc_in1, cc_out1, g1_t, bt1_t,
                                 scale1, bias1, m // GM, "1")
                        sign_group(m // GM)
                emit_signs(len(sign_tasks))

            ph1.__exit__(None, None, None)
            # ================= Phase 2: fc2 + prelu + stats + fc3 ============
            ph2 = nc.named_scope("fc2")
            ph2.__enter__()
            with tc.tile_pool(name="lgs", bufs=1) as lg_pool:
              # logits accumulator: chunk ng at partitions 64*(ng%2)+0..9,
              # cols 512*(ng//2)..+512; outlives the big fc2 pools so the
              # epilogue can run after they release.
              logits_sb = lg_pool.tile([128, 4 * 512], F32, tag="logits")
              with (
                tc.tile_pool(name="s1", bufs=1) as s1_pool,
                tc.tile_pool(name="w2p", bufs=2) as w2_pool,
                tc.tile_pool(name="p2w", bufs=(P2W - 1 if debug else P2W))
                    as p2w_pool,
                tc.tile_pool(name="qp", bufs=3) as q_pool,
                tc.tile_pool(name="ps3", bufs=2, space="PSUM") as ps3_pool,
              ):
                def load_w2(m):
                    w2_t = w2_pool.tile([128, MT, 128], F8, tag="w2",
                                        name=f"w2_{m}")
                    for k0, k1 in ((0, 16), (16, MT)):
                        nc.sync.dma_start(
                            w2_t[:, k0:k1, :], w2.ap()[m][:, k0:k1, :]
                        )
                    return w2_t

                w2_cur = load_w2(0)
                s1_t = s1_pool.tile([128, MT, BS], F8, tag="s1")
                for k in range(MT):
                    nc.sync.dma_start(s1_t[:, k, :], s1d[k])

                p2w_tiles = {}
                fc3_tasks = []

                def fc3_group(g):
                    for ng in range(NB):
                        fc3_tasks.append((g, ng))

                def emit_fc3(k):
                    for _ in range(min(k, len(fc3_tasks))):
                        g, ng = fc3_tasks.pop(0)
                        rp = 64 * (ng % 2)
                        cl = 512 * (ng // 2)
                        ps3 = ps3_pool.tile([128, 512], F32, tag="ps3",
                                            name=f"ps3_{g}_{ng}")
                        for i, mm in enumerate(range(g * GM, (g + 1) * GM)):
                            q = q_pool.tile([128, 512], F16, tag="q",
                                            name=f"q_{g}_{ng}_{i}")
                            nc.vector.tensor_scalar(
                                q[:], p2w_tiles[mm][:, ng * 512:(ng + 1) * 512],
                                scale2[:, mm:mm + 1], bias2[:, mm:mm + 1],
                                ALU.mult, ALU.add,
                            )
                            nc.tensor.matmul(
                                ps3[rp:rp + C, :], w3_t[:, mm, :], q[:],
                                start=(i == 0), stop=(i == GM - 1),
                                tile_position=(0, rp),
                            )
                        if g == 0:
                            nc.vector.tensor_copy(
                                logits_sb[rp:rp + C, cl:cl + 512],
                                ps3[rp:rp + C, :],
                            )
                        else:
                            nc.vector.tensor_add(
                                logits_sb[rp:rp + C, cl:cl + 512],
                                logits_sb[rp:rp + C, cl:cl + 512],
                                ps3[rp:rp + C, :],
                            )

                with tc.tile_pool(name="mm2", bufs=4, space="PSUM") as mm2_pool:
                    for m in range(MT):
                        emit_fc3(3)
                        w2_t = w2_cur
                        if m + 1 < MT:
                            w2_cur = load_w2(m + 1)
                        p2w = p2w_pool.tile([128, BS], F16, tag="p2w",
                                            name=f"p2w_{m}")
                        p2w_tiles[m] = p2w
                        for ng in range(NB):
                            ps = mm2_pool.tile([128, 512], F32, tag="mm2")
                            for kk in range(MT // 2):
                                nc.tensor.matmul(
                                    ps[:],
                                    w2_t[:, 2 * kk:2 * kk + 2, :],
                                    s1_t[:, 2 * kk:2 * kk + 2,
                                         ng * 512:(ng + 1) * 512],
                                    start=(kk == 0),
                                    stop=(kk == MT // 2 - 1),
                                    perf_mode=mybir.MatmulPerfMode.DoubleRow,
                                )
                            nc.scalar.activation(
                                p2w[:, ng * 512:(ng + 1) * 512], ps[:],
                                AF.Prelu, alpha=a2_t[:],
                                accum_out=sums2[:, m, ng:ng + 1],
                            )
                            scr = q_pool.tile([128, 512], F16, tag="scr2",
                                              name=f"scr2_{m}_{ng}")
                            nc.vector.scalar_tensor_tensor(
                                scr[:], p2w[:, ng * 512:(ng + 1) * 512], 0.0,
                                p2w[:, ng * 512:(ng + 1) * 512],
                                ALU.add, ALU.mult,
                                accum_out=sq2[:, m, ng:ng + 1],
                            )
                        if m % GM == GM - 1:
                            bn_group(sums2, sq2, cc_in2, cc_out2, g2_t, bt2_t,
                                     scale2, bias2, m // GM, "2")
                            fc3_group(m // GM)
                    emit_fc3(len(fc3_tasks))

                if debug:
                    d2 = q_pool.tile([128, 512], F16, tag="d2", bufs=1)
                    nc.vector.tensor_copy(d2[:], p2w_tiles[MT - 1][:, 0:512])
                    d2f = q_pool.tile([128, 512], F32, tag="d2f", bufs=1)
                    nc.vector.tensor_copy(d2f[:], d2[:])
                    nc.sync.dma_start(dbg["dbg_p2"].ap(), d2f[:])

              ph2.__exit__(None, None, None)
              # ============ Epilogue: + b3, transpose, log_softmax ===========
              ph3 = nc.named_scope("epi")
              ph3.__enter__()
              with tc.tile_pool(name="epi_dummy", bufs=1) as _epi:
                with (
                    tc.tile_pool(name="lg0", bufs=2) as lg0_pool,
                    tc.tile_pool(name="pst", bufs=4, space="PSUM") as pst_pool,
                    tc.tile_pool(name="sm", bufs=4) as sm_pool,
                    tc.tile_pool(name="op", bufs=4) as out_pool,
                ):
                    for ng in range(NB):
                        rp = 64 * (ng % 2)
                        cl = 512 * (ng // 2)
                        lg0 = lg0_pool.tile([C, 512], F32, tag="lg0",
                                            name=f"lg0_{ng}")
                        nc.sync.dma_start(
                            lg0[:], logits_sb[rp:rp + C, cl:cl + 512]
                        )
                        lg = lg0_pool.tile([C, 512], F32, tag="lg",
                                           name=f"lg_{ng}")
                        nc.scalar.activation(lg[:], lg0[:], AF.Identity,
                                             bias=b3_t[:])
                        for j in range(4):
                            pt = pst_pool.tile([128, C], F32, tag="pt")
                            nc.tensor.transpose(
                                pt[:], lg[:, j * 128:(j + 1) * 128], eye_t[:]
                            )
                            # |logits| <= ~10 so exp() is safe in fp32
                            # without the max-subtraction.
                            ex = sm_pool.tile([128, C], F32, tag="ex")
                            se = sm_pool.tile([128, 1], F32, tag="se")
                            nc.scalar.activation(
                                ex[:], pt[:], AF.Exp, accum_out=se[:]
                            )
                            ln = sm_pool.tile([128, 1], F32, tag="ln")
                            nc.scalar.activation(ln[:], se[:], AF.Ln)
                            ot = out_pool.tile([128, C], F32, tag="ot")
                            nc.vector.tensor_scalar(
                                ot[:], pt[:], ln[:], None, ALU.subtract
                            )
                            nc.sync.dma_start(
                                out.ap()[ng * 512 + j * 128:
                                         ng * 512 + (j + 1) * 128, :],
                                ot[:],
                            )

              ph3.__exit__(None, None, None)
              if True:
                if debug:
                    for nm, t in [("dbg_scale1", scale1), ("dbg_bias1", bias1),
                                  ("dbg_scale2", scale2), ("dbg_bias2", bias2)]:
                        nc.sync.dma_start(dbg[nm].ap(), t[:])
                    nc.sync.dma_start(dbg["dbg_logits"].ap(),
                                      logits_sb[:, 0:2048])

    nc.compile()
    return nc


def prep_inputs(x, W1, b1, a1, g1, beta1, W2, a2, g2, beta2, W3, b3):
    """Host-side layout prep (baseline fp16 hi/lo fc1 packing)."""
    x = np.ascontiguousarray(np.asarray(x, np.float32))
    W1 = np.asarray(W1, np.float32)
    b1 = np.asarray(b1, np.float32)
    W2 = np.asarray(W2, np.float32)
    W3 = np.asarray(W3, np.float32)
    b3 = np.asarray(b3, np.float32)

    S = np.float32(FS)
    xT_aug = np.zeros((D + 1, B), np.float32)
    xT_aug[0:D] = x.T
    xT_aug[D] = 32.0
    w1T_aug = np.zeros((D + 1, H1), np.float32)
    w1T_aug[0:D] = W1.T
    w1T_aug[D] = b1 / 32.0

    xh = xT_aug.astype(np.float16)
    xl = ((xT_aug - xh.astype(np.float32)) * S).astype(np.float16)
    wh = w1T_aug.astype(np.float16)
    whs = (w1T_aug * S).astype(np.float16)
    wls = ((w1T_aug - wh.astype(np.float32)) * S).astype(np.float16)
    KPAD = KC1 * 128
    A = D + 1
    xF = np.zeros((KPAD, B), np.float16)
    xF[0:A] = xh
    xF[A:2 * A] = xh
    xF[2 * A:2 * A + D] = xl[0:D]
    wF = np.zeros((KPAD, H1), np.float16)
    wF[0:A] = whs
    wF[A:2 * A] = wls
    wF[2 * A:2 * A + D] = wh[0:D]
    w1_blk = np.ascontiguousarray(
        wF.reshape(KC1, 128, MT, 128).transpose(2, 1, 0, 3)
    )

    sW2T = np.where(W2 >= 0, np.float32(1), np.float32(-1)).T
    w2_blk = np.ascontiguousarray(
        sW2T.reshape(MT, 128, MT, 128).transpose(2, 1, 0, 3)
    ).astype(ml_dtypes.float8_e4m3)

    w3_blk = np.ascontiguousarray(
        W3.T.reshape(MT, 128, C).transpose(1, 0, 2)
    ).astype(np.float16)

    def feat_layout(v):
        return np.ascontiguousarray(np.asarray(v, np.float32).reshape(MT, 128).T)

    shared = dict(
        w1=w1_blk, w2=w2_blk, w3=w3_blk,
        g1=feat_layout(g1), bt1=feat_layout(beta1),
        g2=feat_layout(g2), bt2=feat_layout(beta2),
        a1p=np.full((128, 1), np.float32(a1), np.float32),
        a2p=np.full((128, 1), np.float32(a2), np.float32),
        b3p=b3.reshape(C, 1).astype(np.float32),
        eye=np.eye(C, dtype=np.float32),
    )
    in_maps = []
    for c in range(NCORES):
        sl = xF[:, c * BS:(c + 1) * BS]
        xs = np.ascontiguousarray(
            sl.reshape(KC1, 128, NB, 512).transpose(1, 2, 0, 3)
        )
        in_maps.append(dict(shared, xT=xs))
    return in_maps


_NC_CACHE = {}


def run(inputs, debug=False, trace=False):
    key = (debug,)
    if key not in _NC_CACHE:
        _NC_CACHE[key] = build_program(debug=debug)
    nc = _NC_CACHE[key]
    in_maps = prep_inputs(**inputs)
    res = run_bass_kernel_spmd(
        nc, in_maps, core_ids=list(range(NCORES)), trace=trace
    )
    outs = np.concatenate([res.results[c]["out"] for c in range(NCORES)], axis=0)
    return outs, res


def kernel(**inputs):
    out, _ = run(inputs)
    return out
